# revision 15
# baseline (speedup 1.0000x reference)
"""BitExpert (BitNet-style MLP) Trainium2 kernel, 8-core data-parallel.

y = bitlinear(silu(bitlinear(x,w1)) * bitlinear(x,w3), w2)
  with per-token int8 activation quant and per-tensor ternary weight quant.

Strategy (8 NeuronCores, SPMD single NEFF):
 - Data-parallel over tokens: each core takes 1024 of 8192 token rows and a
   full copy of the weights in its own HBM.
 - Weights host-pre-transposed (w1t = w1.T etc.); the three per-tensor
   weight scales are computed on the host (offline weight prep); the heavy
   ternarization streams on-device under the matmuls.
 - w1/w3 ternarize is 2 passes (ACT in-place + DVE) via an offset trick:
   store 192 + clip(round(w*s), -1, 1) in bf16; the bf16 RNE cast performs
   the round.  The +192 offset adds 192*rowsum(x_q) per token, removed via
   the per-partition bias of the PSUM-eviction activation.
 - Phase 1 runs HALF-token-group sweeps: 44 half-sweeps of (16 ko x 4 token
   tiles), alternating two 4-bank PSUM groups.  A group's evictions get a
   full half-sweep (~13.7us) to drain instead of being needed immediately,
   and eviction ops are emitted AHEAD of the next piece conversions in the
   ACT/DVE queues, so the PE never waits on PSUM recycling.
 - h3 evictions are written f16 so h = silu(h1)*h3 runs at DVE 2x rate.
 - h tiles are DMA-xbar-transposed into an SBUF-resident hT during phase 1
   and quantized in place per token-half, pipelined into mm2.
 - Phase 3 runs both token halves concurrently with a 4-chunk lag: psums
   A(4 banks)+B(4 banks) accumulate together, w2 chunks die after 2 uses,
   y evictions interleave with the last chunk's matmuls, and the B-lag
   absorbs the late arrival of the second half's h scales at the phase
   boundary (mm2 A-half starts while the last w1/w3 sweep still runs).
 - All matmul arithmetic is exact: int8 activations and (offset) ternary
   weights in bf16/f16, f32 PSUM accumulation of integers well below 2^24.
"""
import numpy as np

import concourse.mybir as mybir
import concourse.tile as tile
from concourse import bass_utils, bacc
from concourse.masks import make_identity

F32 = mybir.dt.float32
F16 = mybir.dt.float16
BF16 = mybir.dt.bfloat16
AX = mybir.AxisListType
OP = mybir.AluOpType
ACTF = mybir.ActivationFunctionType

NCORES = 8
D = 2048           # d_model
H = 5632           # hidden
TOK = 8192         # total tokens
T = TOK // NCORES  # tokens per core (1024)
P = 128
TT = T // P        # token tiles per core (8)
HALF = TT // 2     # token tiles per half-group (4)
HB = 512           # hidden block (phase 1)
NHB = H // HB      # 11
KD = D // P        # 16
KH = H // P        # 44
DB = 512           # d_model output block (phase 3)
NDB = D // DB      # 4
XC = 1024          # x load chunk

MAGIC = 12582912.0             # 1.5 * 2^23
WOFF = 192.0                   # bf16 round-offset for w1/w3 ternary
EPS = 1e-5

# phase 3 pipeline knobs (in chunk-slots of ~1.7us)
BLAG = 2     # B token-half lags A by this many chunk-slots
QLA = 3      # h-quant lookahead
CLA = 2      # w2 chunk convert lookahead
DLA = 10     # w2 chunk dma lookahead


def _build():
    nc = bacc.Bacc("TRN2", target_bir_lowering=False, debug=False,
                   num_devices=NCORES)
    x = nc.dram_tensor("x", [T, D], F32, kind="ExternalInput").ap()
    w1t = nc.dram_tensor("w1t", [D, H], F32, kind="ExternalInput").ap()
    w2t = nc.dram_tensor("w2t", [H, D], F32, kind="ExternalInput").ap()
    w3t = nc.dram_tensor("w3t", [D, H], F32, kind="ExternalInput").ap()
    wsc = nc.dram_tensor("wsc", [1, 8], F32, kind="ExternalInput").ap()
    y = nc.dram_tensor("y", [T, D], F32, kind="ExternalOutput").ap()

    with tile.TileContext(nc) as tc:
        _body(nc, tc, x, w1t, w2t, w3t, wsc, y)
    nc.compile()
    return nc


def _body(nc, tc, x, w1t, w2t, w3t, wsc, y):
    ctxs = []

    def pool(name, bufs, space="SBUF"):
        cm = tc.tile_pool(name=name, bufs=bufs, space=space)
        p = cm.__enter__()
        ctxs.append((name, cm))
        return p

    def close_pool(name):
        for i, (n, cm) in enumerate(ctxs):
            if n == name:
                cm.__exit__(None, None, None)
                ctxs.pop(i)
                return

    singles = pool("singles", 1)
    wload = pool("wload", 8)  # [P, 512] f32 raw weight pieces (ACT in-place)
    wT = pool("wT", 20)        # [P, HB] bf16 offset-ternary w1/w3 pieces
    scal = pool("scal", 4)     # [P, 1]-ish scalars
    h3p = pool("h3p", 1)       # [P, HB] f16 mm3 evictions
    sApool = pool("sApool", 8)
    hbfp = pool("hbf", 3)      # [P, HB] f16 h tiles pre-transpose
    # two explicit PSUM groups of 4 banks each; half-sweeps alternate
    pG = [pool("psumA", 4, space="PSUM"), pool("psumB", 4, space="PSUM")]
    # innermost SBUF pools: released at the phase 1->3 transition (LIFO)
    gload = pool("gload", 4)   # [P, XC] f32 x chunks
    qb = pool("qb", 4)         # bf16 quantized x chunks (tiles 4-7 stay
                               # live until their transposes at h==1)

    # persistent per-token scalars (one column per token tile)
    mh_all = singles.tile([P, TT], F32)
    sx_all = singles.tile([P, TT], F32)
    rx_all = singles.tile([P, TT], F32)
    rs_all = singles.tile([P, TT], F32)   # rowsum(x_q) per token
    al_all = singles.tile([P, TT], F32)
    be_all = singles.tile([P, TT], F32)
    b1_all = singles.tile([P, TT], F32)   # -WOFF*rs*al
    b3_all = singles.tile([P, TT], F32)   # -WOFF*rs*be
    sh_all = singles.tile([P, TT], F32)
    de_all = singles.tile([P, TT], F32)
    cvec = singles.tile([P, 8], F32)      # [c1 c3 c2 _ s1 s3 s2 _]
    sT = singles.tile([P, T], F16)
    ident = singles.tile([P, P], F32)
    make_identity(nc, ident[:])
    identb = singles.tile([P, P], BF16)
    make_identity(nc, identb[:])
    nc.vector.memset(mh_all[:], 0.0)
    woff_ap = singles.tile([P, 1], F32)
    nc.vector.memset(woff_ap[:], WOFF)
    magic_ap = singles.tile([P, 1], F32)
    nc.vector.memset(magic_ap[:], MAGIC)
    negmagic_ap = singles.tile([P, 1], F32)
    nc.vector.memset(negmagic_ap[:], -MAGIC)

    # persistent activations
    xqT = singles.tile([P, KD, T], BF16)
    shr = singles.tile([1, HALF * P], F16)
    hT = singles.tile([P, KH, T], F16)

    # ---------------- preamble: host-computed weight scales -----------
    wrow = singles.tile([1, 8], F32)
    nc.sync.dma_start(wrow[:], wsc)
    nc.gpsimd.partition_broadcast(cvec[:], wrow[:])
    c1, c3, c2 = cvec[:, 0:1], cvec[:, 1:2], cvec[:, 2:3]
    s1c, s3c, s2c = cvec[:, 4:5], cvec[:, 5:6], cvec[:, 6:7]

    # ---------------- x: absmax, quantize, rowsum ----------
    xq_tiles = [[] for _ in range(TT)]
    xt_tiles = [[] for _ in range(TT)]
    xpose_n = [0]

    def emit_xpose(tt, pool_sel=None):
        # transpose x tile tt on the PE: avoids a cold DMA-notification hop
        for cix in range(D // XC):
            xq = xq_tiles[tt][cix]
            for half in range(2):
                psel = pool_sel if pool_sel is not None \
                    else xpose_n[0] % 2
                tq = pG[psel].tile([P, DB], BF16, tag="ps", name="tq")
                xpose_n[0] += 1
                for q in range(4):
                    cq = half * 4 + q
                    nc.tensor.transpose(
                        tq[:, q * P:(q + 1) * P],
                        xq[:, cq * P:(cq + 1) * P], identb[:])
                nc.vector.tensor_copy(
                    xqT[:, cix * 8 + half * 4:cix * 8 + (half + 1) * 4,
                        tt * P:(tt + 1) * P],
                    tq[:].rearrange("p (a b) -> p a b", b=P))

    def emit_x_a(tt):
        # stage A: loads, absmax, per-token scale
        mx = scal.tile([P, 1], F32, tag="mx")
        for cix in range(D // XC):
            xt = gload.tile([P, XC], F32, tag="gld", name=f"xt{cix}")
            # x rides the scalar DMA ring so it streams concurrently with
            # the weight pieces on the sync ring during the ramp
            nc.scalar.dma_start(
                xt[:], x[tt * P:(tt + 1) * P, cix * XC:(cix + 1) * XC])
            xt_tiles[tt].append(xt)
            mc = scal.tile([P, 1], F32, tag="mxc")
            nc.vector.tensor_reduce(mc[:], xt[:], AX.X, OP.max,
                                    apply_absolute_value=True)
            if cix == 0:
                nc.vector.tensor_scalar(mx[:], mc[:], EPS, None, OP.max)
            else:
                nc.vector.tensor_tensor(mx[:], mx[:], mc[:], OP.max)
        rec = scal.tile([P, 1], F32, tag="rec")
        nc.vector.reciprocal(rec[:], mx[:])
        sx = sx_all[:, tt:tt + 1]
        nc.vector.tensor_scalar(sx, rec[:], 127.0, None, OP.mult)
        nc.vector.reciprocal(rx_all[:, tt:tt + 1], sx)

    def emit_x_b(tt):
        # stage B: quantize, rowsum, eviction constants.
        sx = sx_all[:, tt:tt + 1]
        rs = rs_all[:, tt:tt + 1]
        for cix in range(D // XC):
            xt = xt_tiles[tt][cix]
            nc.scalar.activation(xt[:], xt[:], ACTF.Identity,
                                 bias=magic_ap[:, 0:1], scale=sx)
            xq = qb.tile([P, XC], BF16, tag="qb")
            nc.vector.tensor_scalar(xq[:], xt[:], MAGIC, None, OP.subtract)
            rc = scal.tile([P, 1], F32, tag="rsc")
            nc.vector.tensor_reduce(rc[:], xq[:], AX.X, OP.add)
            if cix == 0:
                nc.vector.tensor_copy(rs, rc[:])
            else:
                nc.vector.tensor_tensor(rs, rs, rc[:], OP.add)
            xq_tiles[tt].append(xq)
        cs = slice(tt, tt + 1)
        nc.vector.tensor_tensor(al_all[:, cs], rx_all[:, cs], c1, OP.mult)
        nc.vector.tensor_tensor(be_all[:, cs], rx_all[:, cs], c3, OP.mult)
        nc.vector.tensor_tensor(b1_all[:, cs], rs, al_all[:, cs], OP.mult)
        nc.vector.tensor_scalar(b1_all[:, cs], b1_all[:, cs], -WOFF, None,
                                OP.mult)
        nc.vector.tensor_tensor(b3_all[:, cs], rs, be_all[:, cs], OP.mult)
        nc.vector.tensor_scalar(b3_all[:, cs], b3_all[:, cs], -WOFF, None,
                                OP.mult)

    # ---------------- w1/w3 piece stream (dma and convert split) ------
    # sweeps: (hb0,w1), (hb0,w3), (hb1,w1), ... each set of 16 ko pieces
    # is consumed by two half-sweeps (token tiles 0-3, then 4-7).
    sweeps = []
    for hb in range(NHB):
        sweeps.append((w1t, s1c, hb, 0))
        sweeps.append((w3t, s3c, hb, 1))
    NSW = len(sweeps)

    wf_tiles = {}
    pc_tiles = {}
    wf_pools = [wload]
    wf_rr = [0]

    def piece_dma(si, ko):
        if si >= NSW or (si, ko) in wf_tiles or (si, ko) in pc_tiles:
            return
        wt_ap, scol, hb, _ = sweeps[si]
        pl = wf_pools[wf_rr[0] % len(wf_pools)]
        wf_rr[0] += 1
        wf = pl.tile([P, HB], F32, tag="wf")
        nc.sync.dma_start(wf[:], wt_ap[ko * P:(ko + 1) * P,
                                       hb * HB:(hb + 1) * HB])
        wf_tiles[(si, ko)] = wf

    def piece_conv(si, ko):
        if si >= NSW or (si, ko) in pc_tiles:
            return
        if (si, ko) not in wf_tiles:
            piece_dma(si, ko)
        wt_ap, scol, hb, _ = sweeps[si]
        wf = wf_tiles.pop((si, ko))
        nc.scalar.activation(wf[:], wf[:], ACTF.Identity,
                             bias=woff_ap[:, 0:1], scale=scol)
        pc = wT.tile([P, HB], BF16, tag="wT")
        nc.vector.tensor_scalar(pc[:], wf[:], WOFF + 1.49, WOFF - 1.49,
                                OP.min, OP.max)
        pc_tiles[(si, ko)] = pc

    # ---------------- evictions -------------------------------------
    sA_tiles = [None] * TT
    hs_ps = {}       # half-sweep index -> list of 4 psum tiles

    def emit_sh_batch(half):
        # per-token h scale, then row-form via one PE transpose batch
        tts = range(half * HALF, (half + 1) * HALF)
        for tt in tts:
            cs = slice(tt, tt + 1)
            tmp = scal.tile([P, 1], F32, tag="shtmp")
            nc.vector.tensor_scalar(tmp[:], mh_all[:, cs], EPS, None, OP.max)
            nc.vector.reciprocal(tmp[:], tmp[:])
            nc.vector.tensor_scalar(sh_all[:, cs], tmp[:], 127.0, None,
                                    OP.mult)
            rh = scal.tile([P, 1], F32, tag="rh")
            nc.vector.reciprocal(rh[:], sh_all[:, cs])
            nc.vector.tensor_tensor(de_all[:, cs], rh[:], c2, OP.mult)
        tps = pG[1 - half % 2].tile([1, HALF * P], F32, tag="ps", name="tps")
        for i, tt in enumerate(tts):
            nc.tensor.transpose(tps[:, i * P:(i + 1) * P],
                                sh_all[:, tt:tt + 1], ident[:])
        nc.vector.tensor_copy(shr[:], tps[:])
        for i, tt in enumerate(tts):
            nc.gpsimd.partition_broadcast(sT[:, tt * P:(tt + 1) * P],
                                          shr[:, i * P:(i + 1) * P])

    def evict_w1_half(h):
        # psa -> sA (silu with offset-correcting bias)
        si, half = h // 2, h % 2
        ps = hs_ps.pop(h)
        for i, tt in enumerate(range(half * HALF, (half + 1) * HALF)):
            sA = sApool.tile([P, HB], F16, tag="sA")
            nc.scalar.activation(sA[:], ps[i][:], ACTF.Silu,
                                 bias=b1_all[:, tt:tt + 1],
                                 scale=al_all[:, tt:tt + 1])
            sA_tiles[tt] = sA

    def evict_w3_half(h):
        # psb -> h3 (f16), h = sA*h3 (f16, DVE 2x), absmax, transpose.
        si, half = h // 2, h % 2
        hb = sweeps[si][2]
        last = hb == NHB - 1
        ps = hs_ps.pop(h)
        for i, tt in enumerate(range(half * HALF, (half + 1) * HALF)):
            h3 = h3p.tile([P, HB], F16, tag="h3")
            if tt % 2 == 1:
                nc.vector.tensor_scalar(h3[:], ps[i][:],
                                        be_all[:, tt:tt + 1],
                                        b3_all[:, tt:tt + 1],
                                        OP.mult, OP.add)
            else:
                nc.scalar.activation(h3[:], ps[i][:], ACTF.Identity,
                                     bias=b3_all[:, tt:tt + 1],
                                     scale=be_all[:, tt:tt + 1])
            hbf = hbfp.tile([P, HB], F16, tag="hbf")
            nc.vector.tensor_tensor(hbf[:], sA_tiles[tt][:], h3[:], OP.mult)
            mpart = scal.tile([P, 1], F32, tag="mpart")
            nc.vector.tensor_reduce(mpart[:], hbf[:], AX.X, OP.max,
                                    apply_absolute_value=True)
            nc.vector.tensor_tensor(mh_all[:, tt:tt + 1],
                                    mh_all[:, tt:tt + 1], mpart[:], OP.max)
            dst = hT[:, hb * (HB // P):(hb + 1) * (HB // P),
                     tt * P:(tt + 1) * P]
            # transposes ride the scalar ring: the sync queue stays
            # exclusive to the weight-piece stream
            nc.scalar.dma_start_transpose(dst, hbf[:])
        if last:
            emit_sh_batch(half)

    def evict_half(h):
        if sweeps[h // 2][3] == 0:
            evict_w1_half(h)
        else:
            evict_w3_half(h)

    # ---------------- ramp: x tiles 0-3, piece set 0 ------------------
    # a(t+1) leads b(t) by one: the DVE always has independent work while
    # stage B waits on the cross-engine ACT hop, and the 4-buf gload pool
    # is never re-targeted before its reader is emitted.
    emit_x_a(0)
    for tt in range(HALF):
        if tt + 1 < HALF:
            emit_x_a(tt + 1)
        emit_x_b(tt)
        emit_xpose(tt)
        if tt >= 2:
            for ko in range(4 * (tt - 2), 4 * (tt - 1)):
                piece_conv(0, ko)
        for k in range(4):
            piece_dma(0, 4 * tt + k)
    for ko in range(4, 8):
        piece_conv(0, ko)

    # ---------------- phase 1: half-sweeps --------------------------
    NH = 2 * NSW
    for h in range(NH):
        si, half = h // 2, h % 2
        # evictions of the previous half-sweep FIRST (ahead of piece
        # conversions in the ACT/DVE queues)
        if h > 0:
            evict_half(h - 1)
        if h == 1:
            # tail of the x ramp: transposes for token tile 7 (pG1,
            # just ahead of this half-sweep's own psum allocations)
            for tt in range(7, TT):
                emit_xpose(tt, pool_sel=1)
        if h == 2:
            # the x ramp is done: reclaim its pools for deeper weight
            # staging (kills the piece-DMA convoy at sweep boundaries)
            close_pool("qb")
            close_pool("gload")
            wf_pools.append(pool("wload2", 12))
        # piece conversions for the upcoming window
        if half == 0:
            for ko in range(8, KD):
                piece_conv(si, ko)
        else:
            for ko in range(4):
                piece_conv(si + 1, ko)
        ps = [pG[half].tile([P, HB], F32, tag="ps", name=f"ps{h}_{i}")
              for i in range(HALF)]
        hs_ps[h] = ps
        tts = list(range(half * HALF, (half + 1) * HALF))
        for ko in range(KD):
            # piece DMA stream: set si+1 during A (ko 0-7 at slots 0-7),
            # rest during B
            if h < 2:
                # staging ring is still 8 bufs: split the dma stream
                if half == 0 and ko < 8:
                    piece_dma(si + 1, ko)
                elif half == 1 and ko >= 8:
                    piece_dma(si + 1, ko)
            elif half == 0:
                # deep ring: the whole next set streams during the A half
                piece_dma(si + 1, ko)
            if half == 1 and ko == 8:
                for k2 in range(4, 8):
                    piece_conv(si + 1, k2)
            piece_conv(si, ko)  # no-op if already converted
            pc = pc_tiles[(si, ko)]
            for i, tt in enumerate(tts):
                nc.tensor.matmul(ps[i][:],
                                 xqT[:, ko, tt * P:(tt + 1) * P], pc[:],
                                 start=(ko == 0), stop=(ko == KD - 1))
            if half == 1:
                pc_tiles.pop((si, ko))
            # x ramp stages for token tiles 4-7, interleaved into the
            # first half-sweep's emission (a leads b by one tile); tiles
            # 4-5 transpose mid-sweep via the idle PSUM group
            if h == 0 and ko % 2 == 1:
                step = ko // 2
                if step + 4 < TT:
                    emit_x_a(step + 4)
                if step >= 1 and step + 3 < TT:
                    emit_x_b(step + 3)
                if step >= 2 and step + 2 < 7:
                    emit_xpose(step + 2, pool_sel=1)

    # ---------------- phase 2->3 transition ---------------------------
    evict_half(NH - 1)
    close_pool("wload2")
    wf_pools.pop()
    w2stage = pool("w2stage", 6)
    wf_pools.append(w2stage)
    wc = pool("wc", 5)         # [P, DB] f16 ternary w2 chunks
    qtmp = pool("qtmp", 1)     # [P, DB] f16 h-quant intermediates
    yout = pool("yout", 2)

    wf2_tiles = {}
    wq_tiles = {}

    def t2_dma(g):
        db, hc = divmod(g, KH)
        if g >= NDB * KH or g in wf2_tiles or g in wq_tiles:
            return
        pl = wf_pools[wf_rr[0] % len(wf_pools)]
        wf_rr[0] += 1
        wf = pl.tile([P, DB], F32, tag="wf", name="wf2")
        nc.sync.dma_start(wf[:], w2t[hc * P:(hc + 1) * P,
                                     db * DB:(db + 1) * DB])
        wf2_tiles[g] = wf

    def t2_conv(g):
        if g >= NDB * KH or g in wq_tiles:
            return
        if g not in wf2_tiles:
            t2_dma(g)
        wf = wf2_tiles.pop(g)
        nc.scalar.activation(wf[:], wf[:], ACTF.Identity,
                             bias=magic_ap[:, 0:1], scale=s2c)
        nc.vector.tensor_scalar(wf[:], wf[:], MAGIC + 1.0, MAGIC - 1.0,
                                OP.min, OP.max)
        wq = wc.tile([P, DB], F16, tag="wc")
        if g < KH:
            nc.scalar.activation(wq[:], wf[:], ACTF.Identity,
                                 bias=negmagic_ap[:, 0:1])
        else:
            nc.vector.tensor_scalar(wq[:], wf[:], MAGIC, None, OP.subtract)
        wq_tiles[g] = wq

    def quant_half(hc, half):
        # quantize hT chunk hc, token half `half`, in place (f16 ints)
        if hc >= KH:
            return
        csl = slice(half * HALF * P, (half + 1) * HALF * P)
        tmp = qtmp.tile([P, DB], F16, tag="qtmp")
        nc.vector.tensor_tensor(tmp[:], hT[:, hc, csl], sT[:, csl], OP.mult)
        nc.vector.tensor_scalar(hT[:, hc, csl], tmp[:], MAGIC, MAGIC,
                                OP.add, OP.subtract)

    def emit_y(pstile, db, tt):
        ysb = yout.tile([P, DB], F32)
        if tt % 2 == 1:
            nc.scalar.mul(ysb[:], pstile[:], de_all[:, tt:tt + 1])
        else:
            nc.vector.tensor_scalar(ysb[:], pstile[:],
                                    de_all[:, tt:tt + 1], None, OP.mult)
        dst = y[tt * P:(tt + 1) * P, db * DB:(db + 1) * DB]
        nc.scalar.dma_start(dst, ysb[:])

    # phase 3 preamble: first w2 chunks + first A-half h-quants
    for g in range(DLA):
        t2_dma(g)
    for g in range(CLA):
        t2_conv(g)
    for hc in range(QLA):
        quant_half(hc, 0)

    # ---------------- phase 3: mm2 slot loop --------------------------
    TOTG = NDB * KH
    psA = psB = None
    for s in range(TOTG + BLAG):
        if s < TOTG:
            db, hc = divmod(s, KH)
            if hc == 0:
                psA = [pG[0].tile([P, DB], F32, tag="ps", name=f"ya{db}_{i}")
                       for i in range(HALF)]
            t2_dma(s + DLA)
            t2_conv(s + CLA)
            if db == 0:
                quant_half(hc + QLA, 0)
            wq = wq_tiles[s]
            for i, tt in enumerate(range(HALF)):
                nc.tensor.matmul(psA[i][:],
                                 hT[:, hc, tt * P:(tt + 1) * P], wq[:],
                                 start=(hc == 0), stop=(hc == KH - 1))
                if hc == KH - 1:
                    emit_y(psA[i], db, tt)
        if 1 <= s <= QLA:
            quant_half(s - 1, 1)
        sb_ = s - BLAG
        if sb_ >= 0:
            db, hc = divmod(sb_, KH)
            if hc == 0:
                psB = [pG[1].tile([P, DB], F32, tag="ps", name=f"yb{db}_{i}")
                       for i in range(HALF)]
            if db == 0:
                quant_half(hc + QLA, 1)
            wq = wq_tiles.pop(sb_)  # B is always the chunk's last user
            for i, tt in enumerate(range(HALF, TT)):
                nc.tensor.matmul(psB[i][:],
                                 hT[:, hc, tt * P:(tt + 1) * P], wq[:],
                                 start=(hc == 0), stop=(hc == KH - 1))
                if hc == KH - 1:
                    emit_y(psB[i], db, tt)

    for _, cm in reversed(ctxs):
        cm.__exit__(None, None, None)
    ctxs.clear()


_NC_CACHE = None


def _get_nc():
    global _NC_CACHE
    if _NC_CACHE is None:
        _NC_CACHE = _build()
    return _NC_CACHE


def kernel(x, w1, w2, w3, trace=False):
    x = np.ascontiguousarray(np.asarray(x, dtype=np.float32))
    w1 = np.asarray(w1, dtype=np.float32)
    w2 = np.asarray(w2, dtype=np.float32)
    w3 = np.asarray(w3, dtype=np.float32)
    w1t = np.ascontiguousarray(w1.T)
    w2t = np.ascontiguousarray(w2.T)
    w3t = np.ascontiguousarray(w3.T)
    B, S, Dm = x.shape
    xf = x.reshape(B * S, Dm)

    # per-tensor weight scales (f32, matching the reference formula)
    one = np.float32(1.0)
    wsc = np.zeros((1, 8), dtype=np.float32)
    for i, w in enumerate((w1, w3, w2)):
        c = np.maximum(np.mean(np.abs(w), dtype=np.float32),
                       np.float32(EPS))
        wsc[0, i] = c            # c1, c3, c2
        wsc[0, 4 + i] = one / c  # s1, s3, s2
    wsc[0, 3] = wsc[0, 7] = one

    in_maps = []
    for i in range(NCORES):
        in_maps.append(dict(
            x=np.ascontiguousarray(xf[i * T:(i + 1) * T]),
            w1t=w1t, w2t=w2t, w3t=w3t, wsc=wsc))

    nc = _get_nc()
    res = bass_utils.run_bass_kernel_spmd(
        nc, in_maps, core_ids=list(range(NCORES)),
        trace=trace, trace_cores=[0] if trace else None)
    out = np.concatenate([res.results[i]["y"] for i in range(NCORES)], axis=0)
    if trace:
        kernel.last_results = res
    return out.reshape(B, S, Dm)


# revision 17
# speedup vs baseline: 1.0209x; 1.0209x over previous
"""BitExpert (BitNet-style MLP) Trainium2 kernel, 8-core data-parallel.

y = bitlinear(silu(bitlinear(x,w1)) * bitlinear(x,w3), w2)
  with per-token int8 activation quant and per-tensor ternary weight quant.

Strategy (8 NeuronCores, SPMD single NEFF):
 - Data-parallel over tokens: each core takes 1024 of 8192 token rows and a
   full copy of the weights in its own HBM.
 - Weights host-pre-transposed (w1t = w1.T etc.); the three per-tensor
   weight scales are computed on the host (offline weight prep); the heavy
   ternarization streams on-device under the matmuls.
 - w1/w3 ternarize is 2 passes (ACT in-place + DVE) via an offset trick:
   store 192 + clip(round(w*s), -1, 1) in bf16; the bf16 RNE cast performs
   the round.  The +192 offset adds 192*rowsum(x_q) per token, removed via
   the per-partition bias of the PSUM-eviction activation.
 - Phase 1 runs HALF-token-group sweeps: 44 half-sweeps of (16 ko x 4 token
   tiles), alternating two 4-bank PSUM groups.  A group's evictions get a
   full half-sweep (~13.7us) to drain instead of being needed immediately,
   and eviction ops are emitted AHEAD of the next piece conversions in the
   ACT/DVE queues, so the PE never waits on PSUM recycling.
 - h3 evictions are written f16 so h = silu(h1)*h3 runs at DVE 2x rate.
 - h tiles are DMA-xbar-transposed into an SBUF-resident hT during phase 1
   and quantized in place per token-half, pipelined into mm2.
 - Phase 3 runs both token halves concurrently with a 4-chunk lag: psums
   A(4 banks)+B(4 banks) accumulate together, w2 chunks die after 2 uses,
   y evictions interleave with the last chunk's matmuls, and the B-lag
   absorbs the late arrival of the second half's h scales at the phase
   boundary (mm2 A-half starts while the last w1/w3 sweep still runs).
 - All matmul arithmetic is exact: int8 activations and (offset) ternary
   weights in bf16/f16, f32 PSUM accumulation of integers well below 2^24.
"""
import numpy as np

import concourse.mybir as mybir
import concourse.tile as tile
from concourse import bass_utils, bacc
from concourse.masks import make_identity

F32 = mybir.dt.float32
F16 = mybir.dt.float16
BF16 = mybir.dt.bfloat16
AX = mybir.AxisListType
OP = mybir.AluOpType
ACTF = mybir.ActivationFunctionType

NCORES = 8
D = 2048           # d_model
H = 5632           # hidden
TOK = 8192         # total tokens
T = TOK // NCORES  # tokens per core (1024)
P = 128
TT = T // P        # token tiles per core (8)
HALF = TT // 2     # token tiles per half-group (4)
HB = 512           # hidden block (phase 1)
NHB = H // HB      # 11
KD = D // P        # 16
KH = H // P        # 44
DB = 512           # d_model output block (phase 3)
NDB = D // DB      # 4
XC = 1024          # x load chunk

MAGIC = 12582912.0             # 1.5 * 2^23
WOFF = 192.0                   # bf16 round-offset for w1/w3 ternary
EPS = 1e-5

# phase 3 pipeline knobs (in chunk-slots of ~1.7us)
BLAG = 2     # B token-half lags A by this many chunk-slots
QLA = 4      # h-quant lookahead
CLA = 2      # w2 chunk convert lookahead
DLA = 12     # w2 chunk dma lookahead


def _build():
    nc = bacc.Bacc("TRN2", target_bir_lowering=False, debug=False,
                   num_devices=NCORES)
    x = nc.dram_tensor("x", [T, D], F32, kind="ExternalInput").ap()
    w1t = nc.dram_tensor("w1t", [D, H], F32, kind="ExternalInput").ap()
    w2t = nc.dram_tensor("w2t", [H, D], F32, kind="ExternalInput").ap()
    w3t = nc.dram_tensor("w3t", [D, H], F32, kind="ExternalInput").ap()
    wsc = nc.dram_tensor("wsc", [1, 8], F32, kind="ExternalInput").ap()
    y = nc.dram_tensor("y", [T, D], F32, kind="ExternalOutput").ap()

    with tile.TileContext(nc) as tc:
        _body(nc, tc, x, w1t, w2t, w3t, wsc, y)
    nc.compile()
    return nc


def _body(nc, tc, x, w1t, w2t, w3t, wsc, y):
    ctxs = []

    def pool(name, bufs, space="SBUF"):
        cm = tc.tile_pool(name=name, bufs=bufs, space=space)
        p = cm.__enter__()
        ctxs.append((name, cm))
        return p

    def close_pool(name):
        for i, (n, cm) in enumerate(ctxs):
            if n == name:
                cm.__exit__(None, None, None)
                ctxs.pop(i)
                return

    singles = pool("singles", 1)
    wload = pool("wload", 8)  # [P, 512] f32 raw weight pieces (ACT in-place)
    wT = pool("wT", 20)        # [P, HB] bf16 offset-ternary w1/w3 pieces
    scal = pool("scal", 4)     # [P, 1]-ish scalars
    h3p = pool("h3p", 1)       # [P, HB] f16 mm3 evictions
    sApool = pool("sApool", 8)
    hbfp = pool("hbf", 3)      # [P, HB] f16 h tiles pre-transpose
    # two explicit PSUM groups of 4 banks each; half-sweeps alternate
    pG = [pool("psumA", 4, space="PSUM"), pool("psumB", 4, space="PSUM")]
    # innermost SBUF pools: released at the phase 1->3 transition (LIFO)
    gload = pool("gload", 4)   # [P, XC] f32 x chunks
    qb = pool("qb", 4)         # bf16 quantized x chunks (tiles 4-7 stay
                               # live until their transposes at h==1)

    # persistent per-token scalars (one column per token tile)
    mh_all = singles.tile([P, TT], F32)
    sx_all = singles.tile([P, TT], F32)
    rx_all = singles.tile([P, TT], F32)
    rs_all = singles.tile([P, TT], F32)   # rowsum(x_q) per token
    al_all = singles.tile([P, TT], F32)
    be_all = singles.tile([P, TT], F32)
    b1_all = singles.tile([P, TT], F32)   # -WOFF*rs*al
    b3_all = singles.tile([P, TT], F32)   # -WOFF*rs*be
    sh_all = singles.tile([P, TT], F32)
    de_all = singles.tile([P, TT], F32)
    cvec = singles.tile([P, 8], F32)      # [c1 c3 c2 _ s1 s3 s2 _]
    sT = singles.tile([P, T], F16)
    ident = singles.tile([P, P], F32)
    make_identity(nc, ident[:])
    identb = singles.tile([P, P], BF16)
    make_identity(nc, identb[:])
    nc.vector.memset(mh_all[:], 0.0)
    woff_ap = singles.tile([P, 1], F32)
    nc.vector.memset(woff_ap[:], WOFF)
    magic_ap = singles.tile([P, 1], F32)
    nc.vector.memset(magic_ap[:], MAGIC)
    negmagic_ap = singles.tile([P, 1], F32)
    nc.vector.memset(negmagic_ap[:], -MAGIC)

    # persistent activations
    xqT = singles.tile([P, KD, T], BF16)
    shr = singles.tile([1, HALF * P], F16)
    hT = singles.tile([P, KH, T], F16)

    # ---------------- preamble: host-computed weight scales -----------
    wrow = singles.tile([1, 8], F32)
    nc.sync.dma_start(wrow[:], wsc)
    nc.gpsimd.partition_broadcast(cvec[:], wrow[:])
    c1, c3, c2 = cvec[:, 0:1], cvec[:, 1:2], cvec[:, 2:3]
    s1c, s3c, s2c = cvec[:, 4:5], cvec[:, 5:6], cvec[:, 6:7]

    # ---------------- x: absmax, quantize, rowsum ----------
    xq_tiles = [[] for _ in range(TT)]
    xt_tiles = [[] for _ in range(TT)]
    xpose_n = [0]

    def emit_xpose(tt, pool_sel=None):
        # transpose x tile tt on the PE: avoids a cold DMA-notification hop
        for cix in range(D // XC):
            xq = xq_tiles[tt][cix]
            for half in range(2):
                psel = pool_sel if pool_sel is not None \
                    else xpose_n[0] % 2
                tq = pG[psel].tile([P, DB], BF16, tag="ps", name="tq")
                xpose_n[0] += 1
                for q in range(4):
                    cq = half * 4 + q
                    nc.tensor.transpose(
                        tq[:, q * P:(q + 1) * P],
                        xq[:, cq * P:(cq + 1) * P], identb[:])
                nc.vector.tensor_copy(
                    xqT[:, cix * 8 + half * 4:cix * 8 + (half + 1) * 4,
                        tt * P:(tt + 1) * P],
                    tq[:].rearrange("p (a b) -> p a b", b=P))

    def emit_x_a(tt):
        # stage A: loads, absmax, per-token scale
        mx = scal.tile([P, 1], F32, tag="mx")
        for cix in range(D // XC):
            xt = gload.tile([P, XC], F32, tag="gld", name=f"xt{cix}")
            # x rides the scalar DMA ring so it streams concurrently with
            # the weight pieces on the sync ring during the ramp
            for hx in range(2):
                nc.scalar.dma_start(
                    xt[:, hx * (XC // 2):(hx + 1) * (XC // 2)],
                    x[tt * P:(tt + 1) * P,
                      cix * XC + hx * (XC // 2):
                      cix * XC + (hx + 1) * (XC // 2)])
            xt_tiles[tt].append(xt)
            mc = scal.tile([P, 1], F32, tag="mxc")
            nc.vector.tensor_reduce(mc[:], xt[:], AX.X, OP.max,
                                    apply_absolute_value=True)
            if cix == 0:
                nc.vector.tensor_scalar(mx[:], mc[:], EPS, None, OP.max)
            else:
                nc.vector.tensor_tensor(mx[:], mx[:], mc[:], OP.max)
        rec = scal.tile([P, 1], F32, tag="rec")
        nc.vector.reciprocal(rec[:], mx[:])
        sx = sx_all[:, tt:tt + 1]
        nc.vector.tensor_scalar(sx, rec[:], 127.0, None, OP.mult)
        nc.vector.reciprocal(rx_all[:, tt:tt + 1], sx)

    def emit_x_b(tt):
        # stage B: quantize, rowsum, eviction constants.
        sx = sx_all[:, tt:tt + 1]
        rs = rs_all[:, tt:tt + 1]
        for cix in range(D // XC):
            xt = xt_tiles[tt][cix]
            nc.scalar.activation(xt[:], xt[:], ACTF.Identity,
                                 bias=magic_ap[:, 0:1], scale=sx)
            xq = qb.tile([P, XC], BF16, tag="qb")
            nc.vector.tensor_scalar(xq[:], xt[:], MAGIC, None, OP.subtract)
            rc = scal.tile([P, 1], F32, tag="rsc")
            nc.vector.tensor_reduce(rc[:], xq[:], AX.X, OP.add)
            if cix == 0:
                nc.vector.tensor_copy(rs, rc[:])
            else:
                nc.vector.tensor_tensor(rs, rs, rc[:], OP.add)
            xq_tiles[tt].append(xq)
        cs = slice(tt, tt + 1)
        nc.vector.tensor_tensor(al_all[:, cs], rx_all[:, cs], c1, OP.mult)
        nc.vector.tensor_tensor(be_all[:, cs], rx_all[:, cs], c3, OP.mult)
        nc.vector.tensor_tensor(b1_all[:, cs], rs, al_all[:, cs], OP.mult)
        nc.vector.tensor_scalar(b1_all[:, cs], b1_all[:, cs], -WOFF, None,
                                OP.mult)
        nc.vector.tensor_tensor(b3_all[:, cs], rs, be_all[:, cs], OP.mult)
        nc.vector.tensor_scalar(b3_all[:, cs], b3_all[:, cs], -WOFF, None,
                                OP.mult)

    # ---------------- w1/w3 piece stream (dma and convert split) ------
    # sweeps: (hb0,w1), (hb0,w3), (hb1,w1), ... each set of 16 ko pieces
    # is consumed by two half-sweeps (token tiles 0-3, then 4-7).
    sweeps = []
    for hb in range(NHB):
        sweeps.append((w1t, s1c, hb, 0))
        sweeps.append((w3t, s3c, hb, 1))
    NSW = len(sweeps)

    wf_tiles = {}
    pc_tiles = {}
    wf_pools = [wload]
    wf_rr = [0]

    def piece_dma(si, ko):
        if si >= NSW or (si, ko) in wf_tiles or (si, ko) in pc_tiles:
            return
        wt_ap, scol, hb, _ = sweeps[si]
        pl = wf_pools[wf_rr[0] % len(wf_pools)]
        wf_rr[0] += 1
        wf = pl.tile([P, HB], F32, tag="wf")
        nc.sync.dma_start(wf[:], wt_ap[ko * P:(ko + 1) * P,
                                       hb * HB:(hb + 1) * HB])
        wf_tiles[(si, ko)] = wf

    def piece_conv(si, ko):
        if si >= NSW or (si, ko) in pc_tiles:
            return
        if (si, ko) not in wf_tiles:
            piece_dma(si, ko)
        wt_ap, scol, hb, _ = sweeps[si]
        wf = wf_tiles.pop((si, ko))
        pc = wT.tile([P, HB], BF16, tag="wT")
        # bf16 output cast performs the round (grid 1.0 in [128,256));
        # writing a FRESH tile keeps this op's waits prompt (dma + buf
        # whose readers are matmuls), so it never head-blocks the ACT
        # FIFO in front of the psum-freeing evictions.
        nc.scalar.activation(pc[:], wf[:], ACTF.Identity,
                             bias=woff_ap[:, 0:1], scale=scol)
        nc.vector.tensor_scalar(pc[:], pc[:], WOFF + 1.0, WOFF - 1.0,
                                OP.min, OP.max)
        pc_tiles[(si, ko)] = pc

    # ---------------- evictions -------------------------------------
    sA_tiles = [None] * TT
    hs_ps = {}       # half-sweep index -> list of 4 psum tiles

    def emit_sh_batch(half):
        # per-token h scale, then row-form via one PE transpose batch
        tts = range(half * HALF, (half + 1) * HALF)
        for tt in tts:
            cs = slice(tt, tt + 1)
            tmp = scal.tile([P, 1], F32, tag="shtmp")
            nc.vector.tensor_scalar(tmp[:], mh_all[:, cs], EPS, None, OP.max)
            nc.vector.reciprocal(tmp[:], tmp[:])
            nc.vector.tensor_scalar(sh_all[:, cs], tmp[:], 127.0, None,
                                    OP.mult)
            rh = scal.tile([P, 1], F32, tag="rh")
            nc.vector.reciprocal(rh[:], sh_all[:, cs])
            nc.vector.tensor_tensor(de_all[:, cs], rh[:], c2, OP.mult)
        tps = pG[1 - half % 2].tile([1, HALF * P], F32, tag="ps", name="tps")
        for i, tt in enumerate(tts):
            nc.tensor.transpose(tps[:, i * P:(i + 1) * P],
                                sh_all[:, tt:tt + 1], ident[:])
        nc.vector.tensor_copy(shr[:], tps[:])
        for i, tt in enumerate(tts):
            nc.gpsimd.partition_broadcast(sT[:, tt * P:(tt + 1) * P],
                                          shr[:, i * P:(i + 1) * P])

    def evict_w1_half(h):
        # psa -> sA (silu with offset-correcting bias)
        si, half = h // 2, h % 2
        ps = hs_ps.pop(h)
        for i, tt in enumerate(range(half * HALF, (half + 1) * HALF)):
            sA = sApool.tile([P, HB], F16, tag="sA")
            nc.scalar.activation(sA[:], ps[i][:], ACTF.Silu,
                                 bias=b1_all[:, tt:tt + 1],
                                 scale=al_all[:, tt:tt + 1])
            sA_tiles[tt] = sA

    def evict_w3_half(h):
        # psb -> h3 (f16), h = sA*h3 (f16, DVE 2x), absmax, transpose.
        si, half = h // 2, h % 2
        hb = sweeps[si][2]
        last = hb == NHB - 1
        ps = hs_ps.pop(h)
        for i, tt in enumerate(range(half * HALF, (half + 1) * HALF)):
            h3 = h3p.tile([P, HB], F16, tag="h3")
            if tt % 2 == 1:
                nc.vector.tensor_scalar(h3[:], ps[i][:],
                                        be_all[:, tt:tt + 1],
                                        b3_all[:, tt:tt + 1],
                                        OP.mult, OP.add)
            else:
                nc.scalar.activation(h3[:], ps[i][:], ACTF.Identity,
                                     bias=b3_all[:, tt:tt + 1],
                                     scale=be_all[:, tt:tt + 1])
            hbf = hbfp.tile([P, HB], F16, tag="hbf")
            nc.vector.tensor_tensor(hbf[:], sA_tiles[tt][:], h3[:], OP.mult)
            mpart = scal.tile([P, 1], F32, tag="mpart")
            nc.vector.tensor_reduce(mpart[:], hbf[:], AX.X, OP.max,
                                    apply_absolute_value=True)
            nc.vector.tensor_tensor(mh_all[:, tt:tt + 1],
                                    mh_all[:, tt:tt + 1], mpart[:], OP.max)
            dst = hT[:, hb * (HB // P):(hb + 1) * (HB // P),
                     tt * P:(tt + 1) * P]
            # transposes ride the scalar ring: the sync queue stays
            # exclusive to the weight-piece stream
            nc.scalar.dma_start_transpose(dst, hbf[:])
        if last:
            emit_sh_batch(half)

    def evict_half(h):
        if sweeps[h // 2][3] == 0:
            evict_w1_half(h)
        else:
            evict_w3_half(h)

    # ---------------- ramp: x tiles 0-3, piece set 0 ------------------
    # a(t+1) leads b(t) by one: the DVE always has independent work while
    # stage B waits on the cross-engine ACT hop, and the 4-buf gload pool
    # is never re-targeted before its reader is emitted.
    emit_x_a(0)
    for tt in range(HALF):
        if tt + 1 < HALF:
            emit_x_a(tt + 1)
        emit_x_b(tt)
        emit_xpose(tt)
        if tt >= 2:
            for ko in range(4 * (tt - 2), 4 * (tt - 1)):
                piece_conv(0, ko)
        for k in range(4):
            piece_dma(0, 4 * tt + k)
    for ko in range(4, 8):
        piece_conv(0, ko)

    # ---------------- phase 1: half-sweeps --------------------------
    NH = 2 * NSW
    for h in range(NH):
        si, half = h // 2, h % 2
        # evictions of the previous half-sweep FIRST (ahead of piece
        # conversions in the ACT/DVE queues)
        if h > 0:
            evict_half(h - 1)
        if h == 1:
            # tail of the x ramp: transposes for token tile 7 (pG1,
            # just ahead of this half-sweep's own psum allocations)
            for tt in range(7, TT):
                emit_xpose(tt, pool_sel=1)
        if h == 2:
            # the x ramp is done: reclaim its pools for deeper weight
            # staging (kills the piece-DMA convoy at sweep boundaries)
            close_pool("qb")
            close_pool("gload")
            wf_pools.append(pool("wload2", 12))
            for k2 in range(8):
                piece_dma(si + 1, k2)
        # piece conversions for the upcoming window
        if half == 0:
            for ko in range(8, KD):
                piece_conv(si, ko)
        else:
            for ko in range(4):
                piece_conv(si + 1, ko)
        ps = [pG[half].tile([P, HB], F32, tag="ps", name=f"ps{h}_{i}")
              for i in range(HALF)]
        hs_ps[h] = ps
        tts = list(range(half * HALF, (half + 1) * HALF))
        for ko in range(KD):
            # piece DMA stream: set si+1 during A (ko 0-7 at slots 0-7),
            # rest during B
            if h < 2:
                # staging ring is still 8 bufs: split the dma stream
                if half == 0 and ko < 8:
                    piece_dma(si + 1, ko)
                elif half == 1 and ko >= 8:
                    piece_dma(si + 1, ko)
            elif half == 0 and ko < 8:
                # deep ring: dma 1.5-2.5 half-sweeps ahead of conversion
                # so the ~12us ring transfer + ~10us completion-notify
                # cycle never stalls the ACT/DVE FIFOs
                piece_dma(si + 1, 8 + ko)
            elif half == 1 and ko < 8:
                piece_dma(si + 2, ko)
            if half == 1 and ko == 8:
                for k2 in range(4, 8):
                    piece_conv(si + 1, k2)
            piece_conv(si, ko)  # no-op if already converted
            pc = pc_tiles[(si, ko)]
            for i, tt in enumerate(tts):
                nc.tensor.matmul(ps[i][:],
                                 xqT[:, ko, tt * P:(tt + 1) * P], pc[:],
                                 start=(ko == 0), stop=(ko == KD - 1))
            if half == 1:
                pc_tiles.pop((si, ko))
            # x ramp stages for token tiles 4-7, interleaved into the
            # first half-sweep's emission (a leads b by one tile); tiles
            # 4-5 transpose mid-sweep via the idle PSUM group
            if h == 0 and ko % 2 == 1:
                step = ko // 2
                if step + 4 < TT:
                    emit_x_a(step + 4)
                if step >= 1 and step + 3 < TT:
                    emit_x_b(step + 3)
                if step >= 2 and step + 2 < 7:
                    emit_xpose(step + 2, pool_sel=1)

    # ---------------- phase 2->3 transition ---------------------------
    evict_half(NH - 1)
    close_pool("wload2")
    wf_pools.pop()
    w2stage = pool("w2stage", 4)
    wf_pools.append(w2stage)
    wraw = pool("wraw", 4)     # [P, DB] bf16 offset-ternary w2 rounds
    wc = pool("wc", 5)         # [P, DB] f16 ternary w2 chunks
    qtmp = pool("qtmp", 1)     # [P, DB] f16 h-quant intermediates
    yout = pool("yout", 2)

    wf2_tiles = {}
    wq_tiles = {}

    def t2_dma(g):
        db, hc = divmod(g, KH)
        if g >= NDB * KH or g in wf2_tiles or g in wq_tiles:
            return
        pl = wf_pools[wf_rr[0] % len(wf_pools)]
        wf_rr[0] += 1
        wf = pl.tile([P, DB], F32, tag="wf", name="wf2")
        nc.sync.dma_start(wf[:], w2t[hc * P:(hc + 1) * P,
                                     db * DB:(db + 1) * DB])
        wf2_tiles[g] = wf

    def t2_conv(g):
        if g >= NDB * KH or g in wq_tiles:
            return
        if g not in wf2_tiles:
            t2_dma(g)
        wf = wf2_tiles.pop(g)
        wr = wraw.tile([P, DB], BF16, tag="wraw")
        nc.scalar.activation(wr[:], wf[:], ACTF.Identity,
                             bias=woff_ap[:, 0:1], scale=s2c)
        nc.vector.tensor_scalar(wr[:], wr[:], WOFF + 1.0, WOFF - 1.0,
                                OP.min, OP.max)
        wq = wc.tile([P, DB], F16, tag="wc")
        nc.vector.tensor_scalar(wq[:], wr[:], WOFF, None, OP.subtract)
        wq_tiles[g] = wq

    def quant_half(hc, half):
        # quantize hT chunk hc, token half `half`, in place (f16 ints)
        if hc >= KH:
            return
        csl = slice(half * HALF * P, (half + 1) * HALF * P)
        tmp = qtmp.tile([P, DB], F16, tag="qtmp")
        nc.vector.tensor_tensor(tmp[:], hT[:, hc, csl], sT[:, csl], OP.mult)
        nc.vector.tensor_scalar(hT[:, hc, csl], tmp[:], MAGIC, MAGIC,
                                OP.add, OP.subtract)

    def emit_y(pstile, db, tt):
        ysb = yout.tile([P, DB], F32)
        if tt % 2 == 1:
            nc.scalar.mul(ysb[:], pstile[:], de_all[:, tt:tt + 1])
        else:
            nc.vector.tensor_scalar(ysb[:], pstile[:],
                                    de_all[:, tt:tt + 1], None, OP.mult)
        dst = y[tt * P:(tt + 1) * P, db * DB:(db + 1) * DB]
        nc.scalar.dma_start(dst, ysb[:])

    # phase 3 preamble: first w2 chunks + first A-half h-quants
    for g in range(DLA):
        t2_dma(g)
    for g in range(CLA):
        t2_conv(g)
    for hc in range(QLA):
        quant_half(hc, 0)

    # ---------------- phase 3: mm2 slot loop --------------------------
    TOTG = NDB * KH
    psA = psB = None
    for s in range(TOTG + BLAG):
        if s < TOTG:
            db, hc = divmod(s, KH)
            if hc == 0:
                psA = [pG[0].tile([P, DB], F32, tag="ps", name=f"ya{db}_{i}")
                       for i in range(HALF)]
            if db == 0:
                quant_half(hc + QLA, 0)
            wq = wq_tiles[s]
            for i, tt in enumerate(range(HALF)):
                nc.tensor.matmul(psA[i][:],
                                 hT[:, hc, tt * P:(tt + 1) * P], wq[:],
                                 start=(hc == 0), stop=(hc == KH - 1))
                if hc == KH - 1:
                    emit_y(psA[i], db, tt)
        if 1 <= s <= QLA:
            quant_half(s - 1, 1)
        sb_ = s - BLAG
        if sb_ >= 0:
            db, hc = divmod(sb_, KH)
            if hc == 0:
                psB = [pG[1].tile([P, DB], F32, tag="ps", name=f"yb{db}_{i}")
                       for i in range(HALF)]
            if db == 0:
                quant_half(hc + QLA, 1)
            wq = wq_tiles.pop(sb_)  # B is always the chunk's last user
            for i, tt in enumerate(range(HALF, TT)):
                nc.tensor.matmul(psB[i][:],
                                 hT[:, hc, tt * P:(tt + 1) * P], wq[:],
                                 start=(hc == 0), stop=(hc == KH - 1))
                if hc == KH - 1:
                    emit_y(psB[i], db, tt)
        if s < TOTG:
            t2_dma(s + DLA)
            t2_conv(s + CLA)

    for _, cm in reversed(ctxs):
        cm.__exit__(None, None, None)
    ctxs.clear()


_NC_CACHE = None


def _get_nc():
    global _NC_CACHE
    if _NC_CACHE is None:
        _NC_CACHE = _build()
    return _NC_CACHE


def kernel(x, w1, w2, w3, trace=False):
    x = np.ascontiguousarray(np.asarray(x, dtype=np.float32))
    w1 = np.asarray(w1, dtype=np.float32)
    w2 = np.asarray(w2, dtype=np.float32)
    w3 = np.asarray(w3, dtype=np.float32)
    w1t = np.ascontiguousarray(w1.T)
    w2t = np.ascontiguousarray(w2.T)
    w3t = np.ascontiguousarray(w3.T)
    B, S, Dm = x.shape
    xf = x.reshape(B * S, Dm)

    # per-tensor weight scales (f32, matching the reference formula)
    one = np.float32(1.0)
    wsc = np.zeros((1, 8), dtype=np.float32)
    for i, w in enumerate((w1, w3, w2)):
        c = np.maximum(np.mean(np.abs(w), dtype=np.float32),
                       np.float32(EPS))
        wsc[0, i] = c            # c1, c3, c2
        wsc[0, 4 + i] = one / c  # s1, s3, s2
    wsc[0, 3] = wsc[0, 7] = one

    in_maps = []
    for i in range(NCORES):
        in_maps.append(dict(
            x=np.ascontiguousarray(xf[i * T:(i + 1) * T]),
            w1t=w1t, w2t=w2t, w3t=w3t, wsc=wsc))

    nc = _get_nc()
    res = bass_utils.run_bass_kernel_spmd(
        nc, in_maps, core_ids=list(range(NCORES)),
        trace=trace, trace_cores=[0] if trace else None)
    out = np.concatenate([res.results[i]["y"] for i in range(NCORES)], axis=0)
    if trace:
        kernel.last_results = res
    return out.reshape(B, S, Dm)
